# revision 1
# baseline (speedup 1.0000x reference)
"""AttentionPool (segment softmax + weighted segment sum) on 8 TRN2 cores.

kernel(x, batch, W1, b1, W2, b2) -> pooled [4096, 256] f32

Strategy (data-parallel over graphs, per the sharding hint):
  - batch is sorted, so nodes of each graph are contiguous. Each core gets
    512 consecutive graphs = 16 windows of GW=32 graphs. Each window's nodes
    are padded to a common NW rows (multiple of 512) and laid out so that a
    512-node block DMA reads 2KB contiguous per partition.
  - x is converted to bf16 on host (halves HBM traffic; scores tolerate it
    easily and pooled output error stays ~1e-3).
  - Per 512-node block on device: 8 PE-transposes of x (bf16 in/out), 2
    accumulating MLP matmuls with W1 (K=256), tanh(+b1) on ACT -> th bf16,
    per-chunk transposed score matmuls with W2 (scores node-on-partition),
    exp, one-hot weights oe = (iota==gl)*st built on DVE in bf16.
  - Pooling uses x as the matmul *stationary* operand: per 128-node chunk
    and channel half, acc_T[ch,128x2,gw] += x_chunk^T @ oe (only gw=32
    moving columns instead of 257), and a ones-vector stationary gives the
    softmax denominators acc_d[1,gw]. Both accumulate in PSUM across the
    window's blocks.
  - Finalize per window: PE-transpose acc_T/acc_d back to graph-on-partition
    (independent full-group matmuls into disjoint regions of one PSUM bank),
    pooled = acc / (denom + 1e-16) via per-partition scalars, DMA out fp32.
  - b2 is skipped (softmax invariant); max-subtraction skipped (|score| is
    bounded by ~sum|W2|, exp stays comfortably in fp32 range).

Padding rows carry local-graph-id -1 so their one-hot column is zero: they
contribute to neither numerator nor denominator.
"""

from contextlib import ExitStack

import numpy as np
import ml_dtypes

import concourse.bass as bass
import concourse.mybir as mybir
import concourse.tile as tile
from concourse import bacc, bass_utils
from concourse.masks import make_identity

FP32 = mybir.dt.float32
BF16 = mybir.dt.bfloat16
C = 256
BLK = 512
KCH = BLK // 128
N_CORES = 8
GW = 32  # graphs per window
NWIN = 16  # windows per core -> 512 graphs per core

_NC_CACHE = {}


def _build_nc(nwin, nw, gw, repeat=1):
    # repeat > 1 re-runs the whole computation (same inputs/outputs) inside
    # one NEFF; used only for overhead-cancelling timing measurements.
    assert nw % BLK == 0 and gw <= 128
    bpw = nw // BLK
    nblk = nwin * bpw

    nc = bacc.Bacc(None, target_bir_lowering=False)

    xd = nc.dram_tensor("xd", [nblk, 128, KCH, C], BF16, kind="ExternalInput")
    gl = nc.dram_tensor("gl", [nwin, 128, bpw * KCH], FP32, kind="ExternalInput")
    w1 = nc.dram_tensor("w1", [128, 2 * 128], BF16, kind="ExternalInput")
    b1v = nc.dram_tensor("b1v", [128, 1], FP32, kind="ExternalInput")
    w2 = nc.dram_tensor("w2", [128, 1], BF16, kind="ExternalInput")
    pooled = nc.dram_tensor("pooled", [nwin * gw, C], FP32, kind="ExternalOutput")

    with tile.TileContext(nc) as tc, ExitStack() as ctx:
        consts = ctx.enter_context(tc.tile_pool(name="consts", bufs=1))
        glp = ctx.enter_context(tc.tile_pool(name="glp", bufs=2))
        xp = ctx.enter_context(tc.tile_pool(name="xp", bufs=6))
        xtp = ctx.enter_context(tc.tile_pool(name="xtp", bufs=2))
        thp = ctx.enter_context(tc.tile_pool(name="thp", bufs=2))
        sp = ctx.enter_context(tc.tile_pool(name="sp", bufs=4))
        oep = ctx.enter_context(tc.tile_pool(name="oep", bufs=2))
        outp = ctx.enter_context(tc.tile_pool(name="outp", bufs=2))
        ps_xt = ctx.enter_context(tc.tile_pool(name="ps_xt", bufs=2, space="PSUM"))
        ps_h = ctx.enter_context(tc.tile_pool(name="ps_h", bufs=2, space="PSUM"))
        ps_s = ctx.enter_context(tc.tile_pool(name="ps_s", bufs=1, space="PSUM"))
        ps_acc = ctx.enter_context(tc.tile_pool(name="ps_acc", bufs=2, space="PSUM"))
        ps_fin = ctx.enter_context(tc.tile_pool(name="ps_fin", bufs=1, space="PSUM"))

        w1_sb = consts.tile([128, 2, 128], BF16)
        nc.sync.dma_start(out=w1_sb[:], in_=w1[:].rearrange("p (cb j) -> p cb j", cb=2))
        b1_sb = consts.tile([128, 1], FP32)
        nc.sync.dma_start(out=b1_sb[:], in_=b1v[:])
        w2_sb = consts.tile([128, 1], BF16)
        nc.sync.dma_start(out=w2_sb[:], in_=w2[:])
        ident_b = consts.tile([128, 128], BF16)
        make_identity(nc, ident_b[:])
        ident_f = consts.tile([128, 128], FP32)
        make_identity(nc, ident_f[:])
        ident1 = consts.tile([1, 1], FP32)
        nc.vector.memset(ident1[:], 1.0)
        ones_sb = consts.tile([128, 1], BF16)
        nc.vector.memset(ones_sb[:], 1.0)
        iota_i = consts.tile([128, gw], mybir.dt.int32)
        nc.gpsimd.iota(iota_i[:], pattern=[[1, gw]], base=0, channel_multiplier=0)
        iota_g = consts.tile([128, gw], FP32)
        nc.vector.tensor_copy(out=iota_g[:], in_=iota_i[:])

        # Software pipeline: emit stage A (load, transpose, MLP scores, exp)
        # LOOKAHEAD blocks ahead of stage B (one-hot build + segment
        # matmuls), so the in-order PE stream always has independent
        # transpose/MLP work queued while a block's score chain completes.
        state = {}

        def stage_a(w, b):
            if b == 0:
                state[("gl", w)] = glp.tile([128, bpw * KCH], FP32, name="gl_sb", tag="gl_sb")
                nc.sync.dma_start(out=state[("gl", w)][:], in_=gl[w % nwin])
            xb = xp.tile([128, KCH, C], BF16)
            nc.sync.dma_start(out=xb[:], in_=xd[(w % nwin) * bpw + b])

            xt_ps = ps_xt.tile([128, 2, BLK], BF16)
            for cb in range(2):
                for k in range(KCH):
                    nc.tensor.transpose(
                        out=xt_ps[:, cb, 128 * k : 128 * (k + 1)],
                        in_=xb[:, k, 128 * cb : 128 * (cb + 1)],
                        identity=ident_b[:],
                    )
            xt_sb = xtp.tile([128, 2, BLK], BF16)
            nc.vector.tensor_copy(out=xt_sb[:, 0, :], in_=xt_ps[:, 0, :])
            nc.scalar.copy(out=xt_sb[:, 1, :], in_=xt_ps[:, 1, :])

            h_ps = ps_h.tile([128, BLK], FP32)
            for cb in range(2):
                nc.tensor.matmul(
                    out=h_ps[:],
                    lhsT=w1_sb[:, cb, :],
                    rhs=xt_sb[:, cb, :],
                    start=(cb == 0),
                    stop=(cb == 1),
                )
            th = thp.tile([128, BLK], BF16)
            nc.scalar.activation(
                out=th[:],
                in_=h_ps[:],
                func=mybir.ActivationFunctionType.Tanh,
                bias=b1_sb[:],
                scale=1.0,
            )

            st_ps = ps_s.tile([128, KCH], FP32)
            for k in range(KCH):
                nc.tensor.matmul(
                    out=st_ps[:, k : k + 1],
                    lhsT=th[:, 128 * k : 128 * (k + 1)],
                    rhs=w2_sb[:],
                )
            st = sp.tile([128, KCH], FP32, tag="st")
            nc.scalar.activation(
                out=st[:], in_=st_ps[:], func=mybir.ActivationFunctionType.Exp
            )
            return w, b, xb, st

        def stage_b(w, b, xb, st):
            first = b == 0
            last = b == bpw - 1
            if first:
                # regions: [:, 0:2, :] = pooled^T halves; [0:1, 2, :] = denom.
                # One PSUM bank; the first matmul's start=True clears the
                # bank's has_written bits, later disjoint-region matmuls
                # overwrite-then-accumulate within the same group.
                state[("acc", w)] = ps_acc.tile([128, 6, gw], FP32, name="acc", tag="acc")
            acc = state[("acc", w)]
            gl_sb = state[("gl", w)]

            oe = oep.tile([128, KCH, gw], BF16)
            for k in range(KCH):
                nc.vector.tensor_scalar(
                    out=oe[:, k, :],
                    in0=iota_g[:],
                    scalar1=gl_sb[:, KCH * b + k : KCH * b + k + 1],
                    scalar2=st[:, k : k + 1],
                    op0=mybir.AluOpType.is_equal,
                    op1=mybir.AluOpType.mult,
                )

            for k in range(KCH):
                for cb in range(2):
                    nc.tensor.matmul(
                        out=acc[:, cb, :],
                        lhsT=xb[:, k, 128 * cb : 128 * (cb + 1)],
                        rhs=oe[:, k, :],
                        start=(first and k == 0 and cb == 0),
                        stop=False,
                        skip_group_check=True,
                    )
            nc.tensor.matmul(
                out=acc[0:1, 2:6, :],
                lhsT=ones_sb[:],
                rhs=oe[:],
                start=False,
                stop=last,
                skip_group_check=True,
            )

            if last:
                # PSUM -> SBUF
                accT_sb = outp.tile([128, 2, gw], FP32, tag="accT_sb")
                nc.vector.tensor_copy(out=accT_sb[:], in_=acc[:, 0:2, :])
                d_sb = outp.tile([1, gw], FP32, tag="d_sb")
                nc.vector.tensor_reduce(
                    out=d_sb[:],
                    in_=acc[0:1, 2:6, :].rearrange("p k g -> p g k"),
                    axis=mybir.AxisListType.X,
                    op=mybir.AluOpType.add,
                )
                # transpose back to graph-on-partition; disjoint regions of
                # one PSUM bank (independent full matmul groups)
                fin = ps_fin.tile([gw, 2 * 128 + 1], FP32, name="fin", tag="fin")
                for cb in range(2):
                    nc.tensor.transpose(
                        out=fin[:, 128 * cb : 128 * (cb + 1)],
                        in_=accT_sb[:, cb, :],
                        identity=ident_f[:],
                    )
                nc.tensor.transpose(
                    out=fin[:, 256 : 257],
                    in_=d_sb[:],
                    identity=ident1[:],
                )
                recip = outp.tile([gw, 1], FP32, tag="recip")
                nc.vector.tensor_scalar_add(recip[:], fin[:, 256:257], 1e-16)
                nc.vector.reciprocal(out=recip[:], in_=recip[:])
                out_sb = outp.tile([gw, C], FP32, tag="out_sb")
                nc.vector.tensor_scalar_mul(out_sb[:], fin[:, :256], recip[:])
                nc.sync.dma_start(
                    out=pooled[(w % nwin) * gw : (w % nwin + 1) * gw, :],
                    in_=out_sb[:],
                )

        LOOKAHEAD = 3
        blocks = [(w, b) for w in range(repeat * nwin) for b in range(bpw)]
        pending = []
        for (w, b) in blocks:
            pending.append(stage_a(w, b))
            if len(pending) > LOOKAHEAD:
                stage_b(*pending.pop(0))
        for args in pending:
            stage_b(*args)

    nc.compile()
    return nc


def _shard_inputs(x, batch, W1, b1, W2, nw):
    n_graphs = N_CORES * NWIN * GW
    bpw = nw // BLK
    kj = bpw * KCH
    x = np.asarray(x, dtype=np.float32).astype(ml_dtypes.bfloat16)
    batch = np.asarray(batch)

    wstarts = np.searchsorted(batch, np.arange(0, n_graphs + 1, GW))
    W1 = np.asarray(W1, dtype=np.float32)
    w1_host = np.empty((128, 256), dtype=np.float32)
    for cb in range(2):
        w1_host[:, cb * 128 : (cb + 1) * 128] = W1[cb * 128 : (cb + 1) * 128, :]
    w1_host = w1_host.astype(ml_dtypes.bfloat16)
    b1_host = np.asarray(b1, dtype=np.float32).reshape(128, 1)
    w2_host = np.asarray(W2, dtype=np.float32).astype(ml_dtypes.bfloat16).reshape(128, 1)

    in_maps = []
    for c in range(N_CORES):
        xd = np.zeros((NWIN * bpw, 128, KCH, C), dtype=ml_dtypes.bfloat16)
        gl = np.full((NWIN, 128, kj), -1.0, dtype=np.float32)
        for wl in range(NWIN):
            wg = c * NWIN + wl
            lo, hi = int(wstarts[wg]), int(wstarts[wg + 1])
            cnt = hi - lo
            assert cnt <= nw, f"window {wg} has {cnt} nodes > NW={nw}"
            xpad = np.zeros((nw, C), dtype=ml_dtypes.bfloat16)
            xpad[:cnt] = x[lo:hi]
            xd[wl * bpw : (wl + 1) * bpw] = xpad.reshape(bpw, KCH, 128, C).transpose(
                0, 2, 1, 3
            )
            glpad = np.full((nw,), -1.0, dtype=np.float32)
            glpad[:cnt] = (batch[lo:hi] - wg * GW).astype(np.float32)
            gl[wl] = glpad.reshape(bpw, KCH, 128).transpose(2, 0, 1).reshape(128, kj)
        in_maps.append(
            {"xd": xd, "gl": gl, "w1": w1_host, "b1v": b1_host, "w2": w2_host}
        )
    return in_maps


def kernel(x, batch, W1, b1, W2, b2):
    x = np.asarray(x)
    batch = np.asarray(batch)
    n_graphs = N_CORES * NWIN * GW
    assert x.shape[1] == C and batch.shape[0] == x.shape[0]

    # padded nodes per window, from the actual data
    wstarts = np.searchsorted(batch, np.arange(0, n_graphs + 1, GW))
    max_win = int(np.diff(wstarts).max())
    nw = max(BLK, -(-max_win // BLK) * BLK)

    key = (NWIN, nw, GW)
    if key not in _NC_CACHE:
        _NC_CACHE[key] = _build_nc(*key)
    nc = _NC_CACHE[key]

    in_maps = _shard_inputs(x, batch, W1, b1, W2, nw)
    res = bass_utils.run_bass_kernel_spmd(
        nc,
        in_maps,
        core_ids=list(range(N_CORES)),
    )
    out = np.concatenate(
        [res.results[c]["pooled"] for c in range(N_CORES)], axis=0
    ).astype(np.float32)
    return out

